# revision 7
# baseline (speedup 1.0000x reference)
"""CRF log-likelihood loss kernel for Trainium2 (8 NeuronCores, Bass/Tile).

Strategy (data-parallel over batch, per sharding hint):
  - B=256 batch rows sharded 32 per core; W/b/CRF tables replicated.
  - Host pre-transposes each emissions shard to [H, T, Bs] so the device
    matmul (contract over H on partitions) needs no on-device transposes.
  - Device: logits^T[k, (t,b)] = W^T @ emisT  (PE, PSUM accumulate over 2
    h-chunks); X = exp(logits + bias) (ACT, bias fused); gold-tag logit sum
    via tensor_tensor_reduce against a host-built one-hot (DVE).
  - Forward algorithm in the linear domain: a_t[j,b] stays transposed
    [K, Bs] so each step is ONE matmul with lhsT = exp(transitions)
    augmented with a ones-column (emits column sums for free) plus ONE DVE
    multiply by X_t. Every 8 steps the state is renormalized by the sum row
    (reciprocal + ones-outer-product broadcast matmul + multiply); the norms
    are recorded and folded back in on the host.
  - Host finishes: logZ_b = sum(ln s) + ln(sum_j a_final[j,b]*exp(end_j));
    numerator = device gold-logit sum + tags-only terms (start/trans/end/bias)
    computed on host; final scalar = sum_b(score_b - logZ_b).
"""

import numpy as np

B, T, H, K = 256, 512, 256, 32
NCORES = 8
BS = B // NCORES          # 32 batch rows per core
NT = T * BS               # 16384 tokens per core
CHUNK = 2048              # tokens per DMA chunk
SUB = 512                 # tokens per matmul / X tile
NCHUNK = NT // CHUNK      # 8
NSUB = CHUNK // SUB       # 4
NXT = NT // SUB           # 32 X tiles
TS_PER_XT = SUB // BS     # 16 t-steps per X tile
RENORM = 8                # renormalize the chain state every 8 steps
NRENORM = T // RENORM     # 64 renorms (at t = 7, 15, ..., 511)

_BUILT = {}
LAST_RESULTS = None


def _build_nc(parts="all"):
    import concourse.bacc as bacc
    import concourse.tile as tile
    from concourse import mybir
    from contextlib import ExitStack

    do_bulk = parts in ("all", "bulk", "bulk_nottr", "bulk_ttr2", "bulk_mr")
    do_ttr = parts in ("all", "bulk", "bulk_ttr2", "bulk_mr")
    # TENSOR_TENSOR_REDUCE crashes this HW/FW (NRT_EXEC_UNIT_UNRECOVERABLE,
    # verified by bisection) — use mult + reduce_sum + add instead.
    ttr_mode = {"bulk_ttr2": "ttr2", "bulk": "ttr"}.get(parts, "mr")
    do_chain = parts in ("all", "chain", "chain_norenorm")
    do_renorm = parts in ("all", "chain")

    f32 = mybir.dt.float32
    Exp = mybir.ActivationFunctionType.Exp
    Copy = mybir.ActivationFunctionType.Copy
    mult = mybir.AluOpType.mult
    add = mybir.AluOpType.add

    nc = bacc.Bacc("TRN2", target_bir_lowering=False, debug=False,
                   num_devices=NCORES)

    emisT = nc.declare_dram_parameter("emisT", [2, 128, NT], f32, isOutput=False)
    oht = nc.declare_dram_parameter("oht", [K, NT], f32, isOutput=False)
    wT = nc.declare_dram_parameter("wT", [2, 128, K], f32, isOutput=False)
    ehat = nc.declare_dram_parameter("ehat", [K, K + 1], f32, isOutput=False)
    bvec = nc.declare_dram_parameter("bvec", [K, 1], f32, isOutput=False)
    estart = nc.declare_dram_parameter("estart", [K, 1], f32, isOutput=False)
    afinal_d = nc.declare_dram_parameter("afinal", [K, BS], f32, isOutput=True)
    shist_d = nc.declare_dram_parameter("shist", [1, NRENORM * BS], f32, isOutput=True)
    gold_d = nc.declare_dram_parameter("gold", [K, 1], f32, isOutput=True)

    with ExitStack() as ctx:
        tc = ctx.enter_context(tile.TileContext(nc))
        consts = ctx.enter_context(tc.tile_pool(name="consts", bufs=1))
        emis_pool = ctx.enter_context(tc.tile_pool(name="emis", bufs=3))
        oh_pool = ctx.enter_context(tc.tile_pool(name="oh", bufs=2))
        xpool = ctx.enter_context(tc.tile_pool(name="xp", bufs=NXT))
        apool = ctx.enter_context(tc.tile_pool(name="ap", bufs=3))
        tmppool = ctx.enter_context(tc.tile_pool(name="tp", bufs=2))
        rpool = ctx.enter_context(tc.tile_pool(name="rp", bufs=2))
        scrpool = ctx.enter_context(tc.tile_pool(name="scr", bufs=2))
        psum_l = ctx.enter_context(tc.tile_pool(name="pl", bufs=4, space="PSUM"))
        psum_c = ctx.enter_context(tc.tile_pool(name="pc", bufs=2, space="PSUM"))
        psum_b = ctx.enter_context(tc.tile_pool(name="pb", bufs=2, space="PSUM"))

        # constants
        w0 = consts.tile([128, K], f32)
        w1 = consts.tile([128, K], f32)
        ehat_sb = consts.tile([K, K + 1], f32)
        b_sb = consts.tile([K, 1], f32)
        estart_sb = consts.tile([K, 1], f32)
        ones_sb = consts.tile([1, K], f32)
        shist_sb = consts.tile([1, NRENORM * BS], f32)
        gacc = consts.tile([K, 1], f32)
        nc.sync.dma_start(out=w0, in_=wT[0])
        nc.sync.dma_start(out=w1, in_=wT[1])
        nc.sync.dma_start(out=ehat_sb, in_=ehat[:, :])
        nc.sync.dma_start(out=b_sb, in_=bvec[:, :])
        nc.sync.dma_start(out=estart_sb, in_=estart[:, :])
        nc.vector.memset(ones_sb, 1.0)

        nc.vector.memset(gacc, 0.0)
        nc.vector.memset(shist_sb, 1.0)

        # ---- bulk: logits, X = exp(logits + b), gold-tag logit sum ----
        xtiles = []
        nttr = 0
        for c in range(NCHUNK):
            cs, ce = c * CHUNK, (c + 1) * CHUNK
            if do_bulk:
                e0 = emis_pool.tile([128, CHUNK], f32, tag="e0")
                e1 = emis_pool.tile([128, CHUNK], f32, tag="e1")
                nc.sync.dma_start(out=e0, in_=emisT[0, :, cs:ce])
                nc.sync.dma_start(out=e1, in_=emisT[1, :, cs:ce])
                ohc = oh_pool.tile([K, CHUNK], f32, tag="ohc")
                nc.sync.dma_start(out=ohc, in_=oht[:, cs:ce])
            for s in range(NSUB):
                xt = xpool.tile([K, SUB], f32, tag="xt")
                xtiles.append(xt)
                if not do_bulk:
                    nc.vector.memset(xt, 1.0)
                    continue
                pl = psum_l.tile([K, SUB], f32, tag="pl")
                nc.tensor.matmul(pl, w0, e0[:, s * SUB:(s + 1) * SUB],
                                 start=True, stop=False)
                nc.tensor.matmul(pl, w1, e1[:, s * SUB:(s + 1) * SUB],
                                 start=False, stop=True)
                nc.scalar.activation(out=xt, in_=pl, func=Exp, bias=b_sb)
                if do_ttr:
                    scr = scrpool.tile([K, SUB], f32, tag="scr")
                    ohsl = ohc[:, s * SUB:(s + 1) * SUB]
                    if ttr_mode == "ttr":
                        init = 0.0 if nttr == 0 else gacc
                        nc.vector.tensor_tensor_reduce(
                            out=scr, in0=pl, in1=ohsl,
                            scale=1.0, scalar=init, op0=mult, op1=add,
                            accum_out=gacc)
                    elif ttr_mode == "ttr2":
                        acc_c = rpool.tile([K, 1], f32, tag="acc_c")
                        nc.vector.tensor_tensor_reduce(
                            out=scr, in0=pl, in1=ohsl,
                            scale=1.0, scalar=0.0, op0=mult, op1=add,
                            accum_out=acc_c)
                        nc.vector.tensor_add(gacc, gacc, acc_c)
                    else:
                        acc_c = rpool.tile([K, 1], f32, tag="acc_c")
                        nc.vector.tensor_mul(scr, pl, ohsl)
                        nc.vector.reduce_sum(acc_c, scr,
                                             axis=mybir.AxisListType.X)
                        nc.vector.tensor_add(gacc, gacc, acc_c)
                    nttr += 1

        # ---- chain: linear-domain forward recurrence over t ----
        a_prev = apool.tile([K, BS], f32, tag="a")
        nc.vector.tensor_scalar(out=a_prev, in0=xtiles[0][:, 0:BS],
                                scalar1=estart_sb, scalar2=None, op0=mult)
        if do_chain:
            for t in range(1, T):
                xsl = xtiles[t // TS_PER_XT][:, (t % TS_PER_XT) * BS:
                                             (t % TS_PER_XT + 1) * BS]
                pc = psum_c.tile([K + 1, BS], f32, tag="pc")
                nc.tensor.matmul(pc, ehat_sb, a_prev, start=True, stop=True)
                if do_renorm and t % RENORM == RENORM - 1:
                    slot = t // RENORM
                    rv = rpool.tile([1, BS], f32, tag="rv")
                    nc.vector.reciprocal(rv, pc[K:K + 1, :])
                    nc.scalar.activation(
                        out=shist_sb[0:1, slot * BS:(slot + 1) * BS],
                        in_=pc[K:K + 1, :], func=Copy)
                    pb = psum_b.tile([K, BS], f32, tag="pb")
                    nc.tensor.matmul(pb, ones_sb, rv, start=True, stop=True)
                    atmp = tmppool.tile([K, BS], f32, tag="atmp")
                    nc.vector.tensor_mul(atmp, pc[0:K, :], xsl)
                    a_new = apool.tile([K, BS], f32, tag="a")
                    nc.vector.tensor_mul(a_new, atmp, pb)
                else:
                    a_new = apool.tile([K, BS], f32, tag="a")
                    nc.vector.tensor_mul(a_new, pc[0:K, :], xsl)
                a_prev = a_new

        nc.sync.dma_start(out=afinal_d[:, :], in_=a_prev)
        nc.sync.dma_start(out=shist_d[:, :], in_=shist_sb)
        nc.sync.dma_start(out=gold_d[:, :], in_=gacc)

    nc.compile()
    return nc


def _numpy_fallback(emissions, W, b, start_transitions, transitions,
                    end_transitions, tags, mask):
    # Exact replication of the reference semantics (used only if mask is not
    # all-ones, which the spec's input fill guarantees never happens).
    e = emissions.astype(np.float64)
    logits = e @ W.astype(np.float64) + b.astype(np.float64)
    mf = mask.astype(np.float64)
    st = start_transitions.astype(np.float64)
    tr = transitions.astype(np.float64)
    en = end_transitions.astype(np.float64)
    Bn = logits.shape[0]
    bar = np.arange(Bn)
    first = tags[:, 0]
    score = st[first] + logits[bar, 0, first]
    prev = first.copy()
    for t in range(1, T):
        tg = tags[:, t]
        stepv = tr[prev, tg] + logits[bar, t, tg]
        score = score + stepv * mf[:, t]
        prev = np.where(mf[:, t] > 0, tg, prev)
    score = score + en[prev]
    alpha = st[None, :] + logits[:, 0]
    for t in range(1, T):
        nxt = alpha[:, :, None] + tr[None, :, :]
        m = nxt.max(axis=1, keepdims=True)
        nxt = np.log(np.exp(nxt - m).sum(axis=1)) + m[:, 0, :] + logits[:, t]
        alpha = np.where(mf[:, t:t + 1] > 0, nxt, alpha)
    fin = alpha + en[None, :]
    m = fin.max(axis=1, keepdims=True)
    logz = np.log(np.exp(fin - m).sum(axis=1)) + m[:, 0]
    return np.asarray((score - logz).sum(), dtype=np.float32)


def kernel(emissions, W, b, start_transitions, transitions, end_transitions,
           tags, mask):
    global LAST_RESULTS
    emissions = np.ascontiguousarray(np.asarray(emissions, dtype=np.float32))
    W = np.asarray(W, dtype=np.float32)
    b = np.asarray(b, dtype=np.float32)
    start_transitions = np.asarray(start_transitions, dtype=np.float32)
    transitions = np.asarray(transitions, dtype=np.float32)
    end_transitions = np.asarray(end_transitions, dtype=np.float32)
    tags = np.asarray(tags).astype(np.int64)
    mask = np.asarray(mask).astype(bool)

    if not mask.all():
        return _numpy_fallback(emissions, W, b, start_transitions, transitions,
                               end_transitions, tags, mask)

    from concourse.bass_utils import run_bass_kernel_spmd

    if "nc" not in _BUILT:
        _BUILT["nc"] = _build_nc()
    nc = _BUILT["nc"]

    wT_h = np.ascontiguousarray(W.reshape(2, 128, K))
    ehat_h = np.ascontiguousarray(
        np.concatenate([np.exp(transitions), np.ones((K, 1), np.float32)],
                       axis=1).astype(np.float32))
    bvec_h = np.ascontiguousarray(b.reshape(K, 1))
    estart_h = np.ascontiguousarray(np.exp(start_transitions)
                                    .astype(np.float32).reshape(K, 1))

    in_maps = []
    for c in range(NCORES):
        sh = emissions[c * BS:(c + 1) * BS]              # [BS, T, H]
        emisT_h = np.ascontiguousarray(sh.transpose(2, 1, 0)).reshape(2, 128, NT)
        tg = tags[c * BS:(c + 1) * BS]                   # [BS, T]
        oht_h = np.ascontiguousarray(
            (np.arange(K, dtype=np.int64)[:, None, None] == tg.T[None, :, :])
            .astype(np.float32).reshape(K, NT))
        in_maps.append(dict(emisT=emisT_h, oht=oht_h, wT=wT_h, ehat=ehat_h,
                            bvec=bvec_h, estart=estart_h))

    res = run_bass_kernel_spmd(nc, in_maps, list(range(NCORES)))
    LAST_RESULTS = res

    expend = np.exp(end_transitions.astype(np.float64))
    total = 0.0
    for c in range(NCORES):
        out = res.results[c]
        afinal = out["afinal"].astype(np.float64)        # [K, BS]
        shist = out["shist"].astype(np.float64).reshape(NRENORM, BS)
        gold = out["gold"].astype(np.float64)            # [K, 1]
        logz = np.log(shist).sum(axis=0) + np.log(expend @ afinal)  # [BS]
        tg = tags[c * BS:(c + 1) * BS]
        hterm = (start_transitions.astype(np.float64)[tg[:, 0]].sum()
                 + transitions.astype(np.float64)[tg[:, :-1], tg[:, 1:]].sum()
                 + end_transitions.astype(np.float64)[tg[:, -1]].sum()
                 + b.astype(np.float64)[tg].sum())
        total += gold.sum() + hterm - logz.sum()

    return np.asarray(total, dtype=np.float32)


# revision 21
# speedup vs baseline: 1.0264x; 1.0264x over previous
"""CRF log-likelihood loss kernel for Trainium2 (8 NeuronCores, Bass/Tile).

Strategy (data-parallel over batch, per sharding hint):
  - B=256 batch rows sharded 32 per core; W/b/CRF tables replicated.
  - Host pre-transposes each emissions shard to [H, T, Bs] so the device
    matmul (contract over H on partitions) needs no on-device transposes.
  - Device: logits^T[k, (t,b)] = W^T @ emisT  (PE, PSUM accumulate over 2
    h-chunks); X = exp(logits + bias) (ACT, bias fused); gold-tag logit sum
    via tensor_tensor_reduce against a host-built one-hot (DVE).
  - Forward algorithm in the linear domain: a_t[j,b] stays transposed
    [K, Bs] so each step is ONE matmul with lhsT = exp(transitions)
    augmented with a ones-column (emits column sums for free) plus ONE DVE
    multiply by X_t. Every 8 steps the state is renormalized by the sum row
    (reciprocal + ones-outer-product broadcast matmul + multiply); the norms
    are recorded and folded back in on the host.
  - Host finishes: logZ_b = sum(ln s) + ln(sum_j a_final[j,b]*exp(end_j));
    numerator = device gold-logit sum + tags-only terms (start/trans/end/bias)
    computed on host; final scalar = sum_b(score_b - logZ_b).
"""

import numpy as np

B, T, H, K = 256, 512, 256, 32
NCORES = 8
BS = B // NCORES          # 32 batch rows per core
NT = T * BS               # 16384 tokens per core
CHUNK = 2048              # tokens per DMA chunk
SUB = 512                 # tokens per matmul / X tile
NCHUNK = NT // CHUNK      # 8
NSUB = CHUNK // SUB       # 4
NXT = NT // SUB           # 32 X tiles
TS_PER_XT = SUB // BS     # 16 t-steps per X tile
RENORM = 6                # renormalize the chain state every 6 steps
NRENORM = 84              # measure at t=6m+5, divide at t=6m+11, m=0..83

_BUILT = {}
LAST_RESULTS = None


def _build_nc(parts="all"):
    import concourse.bacc as bacc
    import concourse.tile as tile
    from concourse import mybir
    from contextlib import ExitStack

    import concourse.bass as bass
    from concourse import bass_isa

    do_bulk = parts in ("all", "bulk", "bulk_nottr", "bulk_ttr2", "bulk_mr")
    do_ttr = parts in ("all", "bulk", "bulk_ttr2", "bulk_mr")
    # TENSOR_TENSOR_REDUCE crashes this HW/FW (NRT_EXEC_UNIT_UNRECOVERABLE,
    # verified by bisection) — use mult + reduce_sum + add instead.
    ttr_mode = {"bulk_ttr2": "ttr2", "bulk": "ttr"}.get(parts, "mr")
    do_chain = parts in ("all", "chain", "chain_norenorm")
    do_renorm = parts in ("all", "chain")

    f32 = mybir.dt.float32
    Exp = mybir.ActivationFunctionType.Exp
    Copy = mybir.ActivationFunctionType.Copy
    mult = mybir.AluOpType.mult
    add = mybir.AluOpType.add

    nc = bacc.Bacc("TRN2", target_bir_lowering=False, debug=False,
                   num_devices=NCORES)

    emisT = nc.declare_dram_parameter("emisT", [2, 128, NT], f32, isOutput=False)
    oht = nc.declare_dram_parameter("oht", [K, NT], f32, isOutput=False)
    wT = nc.declare_dram_parameter("wT", [2, 128, K], f32, isOutput=False)
    ehat = nc.declare_dram_parameter("ehat", [K, K], f32, isOutput=False)
    bvec = nc.declare_dram_parameter("bvec", [K, 1], f32, isOutput=False)
    estart = nc.declare_dram_parameter("estart", [K, 1], f32, isOutput=False)
    afinal_d = nc.declare_dram_parameter("afinal", [K, BS], f32, isOutput=True)
    shist_d = nc.declare_dram_parameter("shist", [1, NRENORM * BS], f32, isOutput=True)
    gold_d = nc.declare_dram_parameter("gold", [K, 1], f32, isOutput=True)

    with ExitStack() as ctx:
        tc = ctx.enter_context(tile.TileContext(nc))
        consts = ctx.enter_context(tc.tile_pool(name="consts", bufs=1))
        emis_pool = ctx.enter_context(tc.tile_pool(name="emis", bufs=3))
        oh_pool = ctx.enter_context(tc.tile_pool(name="oh", bufs=2))
        xpool = ctx.enter_context(tc.tile_pool(name="xp", bufs=NXT))
        apool = ctx.enter_context(tc.tile_pool(name="ap", bufs=16))
        tmppool = ctx.enter_context(tc.tile_pool(name="tp", bufs=2))
        rpool = ctx.enter_context(tc.tile_pool(name="rp", bufs=2))
        bcpool = ctx.enter_context(tc.tile_pool(name="bc", bufs=3))
        scrpool = ctx.enter_context(tc.tile_pool(name="scr", bufs=2))
        psum_l = ctx.enter_context(tc.tile_pool(name="pl", bufs=4, space="PSUM"))
        psum_c = ctx.enter_context(tc.tile_pool(name="pc", bufs=3, space="PSUM"))

        # constants
        w0 = consts.tile([128, K], f32)
        w1 = consts.tile([128, K], f32)
        ehat_sb = consts.tile([K, K], f32)
        b_sb = consts.tile([K, 1], f32)
        estart_sb = consts.tile([K, 1], f32)
        shist_sb = consts.tile([1, NRENORM * BS], f32)
        gacc = consts.tile([K, 1], f32)
        nc.sync.dma_start(out=w0, in_=wT[0])
        nc.sync.dma_start(out=w1, in_=wT[1])
        nc.sync.dma_start(out=ehat_sb, in_=ehat[:, :])
        nc.sync.dma_start(out=b_sb, in_=bvec[:, :])
        nc.sync.dma_start(out=estart_sb, in_=estart[:, :])

        nc.vector.memset(gacc, 0.0)
        nc.vector.memset(shist_sb, 1.0)

        # ---- bulk: logits, X = exp(logits + b), gold-tag logit sum ----
        xtiles = []
        nttr = 0
        for c in range(NCHUNK):
            cs, ce = c * CHUNK, (c + 1) * CHUNK
            if do_bulk:
                e0 = emis_pool.tile([128, CHUNK], f32, tag="e0")
                e1 = emis_pool.tile([128, CHUNK], f32, tag="e1")
                nc.sync.dma_start(out=e0, in_=emisT[0, :, cs:ce])
                nc.sync.dma_start(out=e1, in_=emisT[1, :, cs:ce])
                ohc = oh_pool.tile([K, CHUNK], f32, tag="ohc")
                nc.sync.dma_start(out=ohc, in_=oht[:, cs:ce])
            for s in range(NSUB):
                xt = xpool.tile([K, SUB], f32, tag="xt")
                xtiles.append(xt)
                if not do_bulk:
                    nc.vector.memset(xt, 1.0)
                    continue
                pl = psum_l.tile([K, SUB], f32, tag="pl")
                nc.tensor.matmul(pl, w0, e0[:, s * SUB:(s + 1) * SUB],
                                 start=True, stop=False)
                nc.tensor.matmul(pl, w1, e1[:, s * SUB:(s + 1) * SUB],
                                 start=False, stop=True)
                nc.scalar.activation(out=xt, in_=pl, func=Exp, bias=b_sb)
                if do_ttr:
                    scr = scrpool.tile([K, SUB], f32, tag="scr")
                    ohsl = ohc[:, s * SUB:(s + 1) * SUB]
                    if ttr_mode == "ttr":
                        init = 0.0 if nttr == 0 else gacc
                        nc.vector.tensor_tensor_reduce(
                            out=scr, in0=pl, in1=ohsl,
                            scale=1.0, scalar=init, op0=mult, op1=add,
                            accum_out=gacc)
                    elif ttr_mode == "ttr2":
                        acc_c = rpool.tile([K, 1], f32, tag="acc_c")
                        nc.vector.tensor_tensor_reduce(
                            out=scr, in0=pl, in1=ohsl,
                            scale=1.0, scalar=0.0, op0=mult, op1=add,
                            accum_out=acc_c)
                        nc.vector.tensor_add(gacc, gacc, acc_c)
                    else:
                        acc_c = rpool.tile([K, 1], f32, tag="acc_c")
                        nc.vector.tensor_mul(scr, pl, ohsl)
                        nc.vector.reduce_sum(acc_c, scr,
                                             axis=mybir.AxisListType.X)
                        nc.vector.tensor_add(gacc, gacc, acc_c)
                    nttr += 1

        # ---- chain: linear-domain forward recurrence over t ----
        a_prev = apool.tile([K, BS], f32, tag="a")
        nc.vector.tensor_scalar(out=a_prev, in0=xtiles[0][:, 0:BS],
                                scalar1=estart_sb, scalar2=None, op0=mult)
        if do_chain:
            bc_queue = []
            nmeas = 0
            for t in range(1, T):
                xsl = xtiles[t // TS_PER_XT][:, (t % TS_PER_XT) * BS:
                                             (t % TS_PER_XT + 1) * BS]
                pc = psum_c.tile([K, BS], f32, tag="pc")
                nc.tensor.matmul(pc, ehat_sb, a_prev, start=True, stop=True)
                is_apply = do_renorm and t % RENORM == 5 and bc_queue
                if is_apply:
                    # lagged renorm: divide by the column sums measured at
                    # t-6 (column scaling commutes through the chain)
                    atmp = tmppool.tile([K, BS], f32, tag="atmp")
                    nc.vector.tensor_mul(atmp, pc, xsl)
                    a_new = apool.tile([K, BS], f32, tag="a")
                    nc.vector.tensor_mul(a_new, atmp, bc_queue.pop(0))
                else:
                    a_new = apool.tile([K, BS], f32, tag="a")
                    nc.vector.tensor_mul(a_new, pc, xsl)
                a_prev = a_new
                if do_renorm and t % RENORM == 5 and nmeas < NRENORM:
                    # measure column sums of a_t on the (otherwise idle)
                    # GPSIMD engine; partition_all_reduce leaves the sums
                    # replicated on every partition, directly usable as the
                    # divisor 6 steps later
                    slot = nmeas
                    nmeas += 1
                    bc = bcpool.tile([K, BS], f32, tag="bc")
                    nc.gpsimd.partition_all_reduce(
                        bc, a_prev, channels=K, reduce_op=bass_isa.ReduceOp.add)
                    nc.scalar.activation(
                        out=shist_sb[0:1, slot * BS:(slot + 1) * BS],
                        in_=bc[0:1, :], func=Copy)
                    rbc = bcpool.tile([K, BS], f32, tag="rbc")
                    nc.vector.reciprocal(rbc, bc)
                    bc_queue.append(rbc)

        nc.sync.dma_start(out=afinal_d[:, :], in_=a_prev)
        nc.sync.dma_start(out=shist_d[:, :], in_=shist_sb)
        nc.sync.dma_start(out=gold_d[:, :], in_=gacc)

    nc.compile()
    return nc


def _numpy_fallback(emissions, W, b, start_transitions, transitions,
                    end_transitions, tags, mask):
    # Exact replication of the reference semantics (used only if mask is not
    # all-ones, which the spec's input fill guarantees never happens).
    e = emissions.astype(np.float64)
    logits = e @ W.astype(np.float64) + b.astype(np.float64)
    mf = mask.astype(np.float64)
    st = start_transitions.astype(np.float64)
    tr = transitions.astype(np.float64)
    en = end_transitions.astype(np.float64)
    Bn = logits.shape[0]
    bar = np.arange(Bn)
    first = tags[:, 0]
    score = st[first] + logits[bar, 0, first]
    prev = first.copy()
    for t in range(1, T):
        tg = tags[:, t]
        stepv = tr[prev, tg] + logits[bar, t, tg]
        score = score + stepv * mf[:, t]
        prev = np.where(mf[:, t] > 0, tg, prev)
    score = score + en[prev]
    alpha = st[None, :] + logits[:, 0]
    for t in range(1, T):
        nxt = alpha[:, :, None] + tr[None, :, :]
        m = nxt.max(axis=1, keepdims=True)
        nxt = np.log(np.exp(nxt - m).sum(axis=1)) + m[:, 0, :] + logits[:, t]
        alpha = np.where(mf[:, t:t + 1] > 0, nxt, alpha)
    fin = alpha + en[None, :]
    m = fin.max(axis=1, keepdims=True)
    logz = np.log(np.exp(fin - m).sum(axis=1)) + m[:, 0]
    return np.asarray((score - logz).sum(), dtype=np.float32)


def kernel(emissions, W, b, start_transitions, transitions, end_transitions,
           tags, mask):
    global LAST_RESULTS
    emissions = np.ascontiguousarray(np.asarray(emissions, dtype=np.float32))
    W = np.asarray(W, dtype=np.float32)
    b = np.asarray(b, dtype=np.float32)
    start_transitions = np.asarray(start_transitions, dtype=np.float32)
    transitions = np.asarray(transitions, dtype=np.float32)
    end_transitions = np.asarray(end_transitions, dtype=np.float32)
    tags = np.asarray(tags).astype(np.int64)
    mask = np.asarray(mask).astype(bool)

    if not mask.all():
        return _numpy_fallback(emissions, W, b, start_transitions, transitions,
                               end_transitions, tags, mask)

    from concourse.bass_utils import run_bass_kernel_spmd

    if "nc" not in _BUILT:
        _BUILT["nc"] = _build_nc()
    nc = _BUILT["nc"]

    wT_h = np.ascontiguousarray(W.reshape(2, 128, K))
    ehat_h = np.ascontiguousarray(np.exp(transitions).astype(np.float32))
    bvec_h = np.ascontiguousarray(b.reshape(K, 1))
    estart_h = np.ascontiguousarray(np.exp(start_transitions)
                                    .astype(np.float32).reshape(K, 1))

    in_maps = []
    for c in range(NCORES):
        sh = emissions[c * BS:(c + 1) * BS]              # [BS, T, H]
        emisT_h = np.ascontiguousarray(sh.transpose(2, 1, 0)).reshape(2, 128, NT)
        tg = tags[c * BS:(c + 1) * BS]                   # [BS, T]
        oht_h = np.ascontiguousarray(
            (np.arange(K, dtype=np.int64)[:, None, None] == tg.T[None, :, :])
            .astype(np.float32).reshape(K, NT))
        in_maps.append(dict(emisT=emisT_h, oht=oht_h, wT=wT_h, ehat=ehat_h,
                            bvec=bvec_h, estart=estart_h))

    res = run_bass_kernel_spmd(nc, in_maps, list(range(NCORES)))
    LAST_RESULTS = res

    expend = np.exp(end_transitions.astype(np.float64))
    total = 0.0
    for c in range(NCORES):
        out = res.results[c]
        afinal = out["afinal"].astype(np.float64)        # [K, BS]
        shist = out["shist"].astype(np.float64).reshape(NRENORM, BS)
        gold = out["gold"].astype(np.float64)            # [K, 1]
        logz = np.log(shist).sum(axis=0) + np.log(expend @ afinal)  # [BS]
        tg = tags[c * BS:(c + 1) * BS]
        hterm = (start_transitions.astype(np.float64)[tg[:, 0]].sum()
                 + transitions.astype(np.float64)[tg[:, :-1], tg[:, 1:]].sum()
                 + end_transitions.astype(np.float64)[tg[:, -1]].sum()
                 + b.astype(np.float64)[tg].sum())
        total += gold.sum() + hterm - logz.sum()

    return np.asarray(total, dtype=np.float32)


# revision 28
# speedup vs baseline: 1.2921x; 1.2588x over previous
"""CRF log-likelihood loss kernel for Trainium2 (8 NeuronCores, Bass/Tile).

Strategy (data-parallel over batch, per sharding hint):
  - B=256 batch rows sharded 32 per core; W/b/CRF tables replicated.
  - Host pre-transposes each emissions shard to [H, T, Bs] so the device
    matmul (contract over H on partitions) needs no on-device transposes.
  - Device: logits^T[k, (t,b)] = W^T @ emisT  (PE, PSUM accumulate over 2
    h-chunks); X = exp(logits + bias) (ACT, bias fused); gold-tag logit sum
    via tensor_tensor_reduce against a host-built one-hot (DVE).
  - Forward algorithm in the linear domain: a_t[j,b] stays transposed
    [K, Bs] so each step is ONE matmul with lhsT = exp(transitions)
    augmented with a ones-column (emits column sums for free) plus ONE DVE
    multiply by X_t. Every 8 steps the state is renormalized by the sum row
    (reciprocal + ones-outer-product broadcast matmul + multiply); the norms
    are recorded and folded back in on the host.
  - Host finishes: logZ_b = sum(ln s) + ln(sum_j a_final[j,b]*exp(end_j));
    numerator = device gold-logit sum + tags-only terms (start/trans/end/bias)
    computed on host; final scalar = sum_b(score_b - logZ_b).
"""

import numpy as np

B, T, H, K = 256, 512, 256, 32
NCORES = 8
BS = B // NCORES          # 32 batch rows per core
NT = T * BS               # 16384 tokens per core
CHUNK = 2048              # tokens per DMA chunk
SUB = 512                 # tokens per matmul / X tile
NCHUNK = NT // CHUNK      # 8
NSUB = CHUNK // SUB       # 4
NXT = NT // SUB           # 32 X tiles
TS_PER_XT = SUB // BS     # 16 t-steps per X tile
RENORM = 6                # renormalize the chain state every 6 steps
NRENORM = 84              # measure at t=6m+5, divide at t=6m+11, m=0..83

_BUILT = {}
LAST_RESULTS = None


def _build_nc(parts="all"):
    import concourse.bacc as bacc
    import concourse.tile as tile
    from concourse import mybir
    from contextlib import ExitStack

    import concourse.bass as bass
    from concourse import bass_isa

    do_bulk = parts in ("all", "bulk", "bulk_nottr", "bulk_ttr2", "bulk_mr")
    do_ttr = parts in ("all", "bulk", "bulk_ttr2", "bulk_mr")
    # TENSOR_TENSOR_REDUCE crashes this HW/FW (NRT_EXEC_UNIT_UNRECOVERABLE,
    # verified by bisection) — use mult + reduce_sum + add instead.
    ttr_mode = {"bulk_ttr2": "ttr2", "bulk": "ttr"}.get(parts, "mr")
    do_chain = parts in ("all", "chain", "chain_norenorm")
    do_renorm = parts in ("all", "chain")

    f32 = mybir.dt.float32
    bf16 = mybir.dt.bfloat16
    Exp = mybir.ActivationFunctionType.Exp
    Copy = mybir.ActivationFunctionType.Copy
    mult = mybir.AluOpType.mult
    add = mybir.AluOpType.add

    nc = bacc.Bacc("TRN2", target_bir_lowering=False, debug=False,
                   num_devices=NCORES)

    emisT = nc.declare_dram_parameter("emisT", [2, 128, NT], f32, isOutput=False)
    oht = nc.declare_dram_parameter("oht", [K, NT], f32, isOutput=False)
    wT = nc.declare_dram_parameter("wT", [2, 128, K], f32, isOutput=False)
    ehat = nc.declare_dram_parameter("ehat", [K, K], bf16, isOutput=False)
    bvec = nc.declare_dram_parameter("bvec", [K, 1], f32, isOutput=False)
    estart = nc.declare_dram_parameter("estart", [K, 1], f32, isOutput=False)
    afinal_d = nc.declare_dram_parameter("afinal", [K, BS], f32, isOutput=True)
    shist_d = nc.declare_dram_parameter("shist", [1, NRENORM * BS], f32, isOutput=True)
    gold_d = nc.declare_dram_parameter("gold", [K, 1], f32, isOutput=True)

    with ExitStack() as ctx:
        tc = ctx.enter_context(tile.TileContext(nc))
        consts = ctx.enter_context(tc.tile_pool(name="consts", bufs=1))
        emis_pool = ctx.enter_context(tc.tile_pool(name="emis", bufs=3))
        oh_pool = ctx.enter_context(tc.tile_pool(name="oh", bufs=2))
        xpool = ctx.enter_context(tc.tile_pool(name="xp", bufs=NXT))
        apool = ctx.enter_context(tc.tile_pool(name="ap", bufs=16))
        tmppool = ctx.enter_context(tc.tile_pool(name="tp", bufs=2))
        rpool = ctx.enter_context(tc.tile_pool(name="rp", bufs=2))
        bcpool = ctx.enter_context(tc.tile_pool(name="bc", bufs=3))
        scrpool = ctx.enter_context(tc.tile_pool(name="scr", bufs=2))
        psum_l = ctx.enter_context(tc.tile_pool(name="pl", bufs=4, space="PSUM"))
        psum_c = ctx.enter_context(tc.tile_pool(name="pc", bufs=3, space="PSUM"))

        # constants
        w0 = consts.tile([128, K], f32)
        w1 = consts.tile([128, K], f32)
        ehat_sb = consts.tile([K, K], bf16)
        b_sb = consts.tile([K, 1], f32)
        estart_sb = consts.tile([K, 1], f32)
        shist_sb = consts.tile([1, NRENORM * BS], f32)
        gacc = consts.tile([K, 1], f32)
        nc.sync.dma_start(out=w0, in_=wT[0])
        nc.sync.dma_start(out=w1, in_=wT[1])
        nc.sync.dma_start(out=ehat_sb, in_=ehat[:, :])
        nc.sync.dma_start(out=b_sb, in_=bvec[:, :])
        nc.sync.dma_start(out=estart_sb, in_=estart[:, :])

        nc.vector.memset(gacc, 0.0)
        nc.vector.memset(shist_sb, 1.0)

        # ---- bulk: logits, X = exp(logits + b), gold-tag logit sum ----
        xtiles = []
        nttr = 0
        for c in range(NCHUNK):
            cs, ce = c * CHUNK, (c + 1) * CHUNK
            if do_bulk:
                e0 = emis_pool.tile([128, CHUNK], f32, tag="e0")
                e1 = emis_pool.tile([128, CHUNK], f32, tag="e1")
                nc.sync.dma_start(out=e0, in_=emisT[0, :, cs:ce])
                nc.sync.dma_start(out=e1, in_=emisT[1, :, cs:ce])
                ohc = oh_pool.tile([K, CHUNK], f32, tag="ohc")
                nc.sync.dma_start(out=ohc, in_=oht[:, cs:ce])
            for s in range(NSUB):
                xt = xpool.tile([K, SUB], f32, tag="xt")
                xtiles.append(xt)
                if not do_bulk:
                    nc.vector.memset(xt, 1.0)
                    continue
                pl = psum_l.tile([K, SUB], f32, tag="pl")
                nc.tensor.matmul(pl, w0, e0[:, s * SUB:(s + 1) * SUB],
                                 start=True, stop=False)
                nc.tensor.matmul(pl, w1, e1[:, s * SUB:(s + 1) * SUB],
                                 start=False, stop=True)
                nc.scalar.activation(out=xt, in_=pl, func=Exp, bias=b_sb)
                if do_ttr:
                    scr = scrpool.tile([K, SUB], f32, tag="scr")
                    ohsl = ohc[:, s * SUB:(s + 1) * SUB]
                    if ttr_mode == "ttr":
                        init = 0.0 if nttr == 0 else gacc
                        nc.vector.tensor_tensor_reduce(
                            out=scr, in0=pl, in1=ohsl,
                            scale=1.0, scalar=init, op0=mult, op1=add,
                            accum_out=gacc)
                    elif ttr_mode == "ttr2":
                        acc_c = rpool.tile([K, 1], f32, tag="acc_c")
                        nc.vector.tensor_tensor_reduce(
                            out=scr, in0=pl, in1=ohsl,
                            scale=1.0, scalar=0.0, op0=mult, op1=add,
                            accum_out=acc_c)
                        nc.vector.tensor_add(gacc, gacc, acc_c)
                    else:
                        acc_c = rpool.tile([K, 1], f32, tag="acc_c")
                        nc.vector.tensor_mul(scr, pl, ohsl)
                        nc.vector.reduce_sum(acc_c, scr,
                                             axis=mybir.AxisListType.X)
                        nc.vector.tensor_add(gacc, gacc, acc_c)
                    nttr += 1

        # ---- chain: linear-domain forward recurrence over t (bf16 state:
        # single-pass PE matmuls; fp32 needs 2 half-speed passes) ----
        a_prev = apool.tile([K, BS], bf16, tag="a")
        nc.vector.tensor_scalar(out=a_prev, in0=xtiles[0][:, 0:BS],
                                scalar1=estart_sb, scalar2=None, op0=mult)
        if do_chain:
            bc_queue = []
            nmeas = 0
            for t in range(1, T):
                xsl = xtiles[t // TS_PER_XT][:, (t % TS_PER_XT) * BS:
                                             (t % TS_PER_XT + 1) * BS]
                pc = psum_c.tile([K, BS], f32, tag="pc")
                nc.tensor.matmul(pc, ehat_sb, a_prev, start=True, stop=True)
                is_apply = do_renorm and t % RENORM == 5 and bc_queue
                if is_apply:
                    # lagged renorm: divide by the column sums measured at
                    # t-6 (column scaling commutes through the chain)
                    atmp = tmppool.tile([K, BS], f32, tag="atmp")
                    nc.vector.tensor_mul(atmp, pc, xsl)
                    a_new = apool.tile([K, BS], bf16, tag="a")
                    nc.vector.tensor_mul(a_new, atmp, bc_queue.pop(0))
                else:
                    a_new = apool.tile([K, BS], bf16, tag="a")
                    nc.vector.tensor_mul(a_new, pc, xsl)
                a_prev = a_new
                if do_renorm and t % RENORM == 5 and nmeas < NRENORM:
                    # measure column sums of a_t on the (otherwise idle)
                    # GPSIMD engine; partition_all_reduce leaves the sums
                    # replicated on every partition, directly usable as the
                    # divisor 6 steps later
                    slot = nmeas
                    nmeas += 1
                    bc = bcpool.tile([K, BS], f32, tag="bc")
                    nc.gpsimd.partition_all_reduce(
                        bc, a_prev, channels=K, reduce_op=bass_isa.ReduceOp.add)
                    nc.scalar.activation(
                        out=shist_sb[0:1, slot * BS:(slot + 1) * BS],
                        in_=bc[0:1, :], func=Copy)
                    rbc = bcpool.tile([K, BS], f32, tag="rbc")
                    nc.vector.reciprocal(rbc, bc)
                    bc_queue.append(rbc)

        nc.gpsimd.dma_start(out=afinal_d[:, :], in_=a_prev)
        nc.sync.dma_start(out=shist_d[:, :], in_=shist_sb)
        nc.sync.dma_start(out=gold_d[:, :], in_=gacc)

    nc.compile()
    return nc


def _numpy_fallback(emissions, W, b, start_transitions, transitions,
                    end_transitions, tags, mask):
    # Exact replication of the reference semantics (used only if mask is not
    # all-ones, which the spec's input fill guarantees never happens).
    e = emissions.astype(np.float64)
    logits = e @ W.astype(np.float64) + b.astype(np.float64)
    mf = mask.astype(np.float64)
    st = start_transitions.astype(np.float64)
    tr = transitions.astype(np.float64)
    en = end_transitions.astype(np.float64)
    Bn = logits.shape[0]
    bar = np.arange(Bn)
    first = tags[:, 0]
    score = st[first] + logits[bar, 0, first]
    prev = first.copy()
    for t in range(1, T):
        tg = tags[:, t]
        stepv = tr[prev, tg] + logits[bar, t, tg]
        score = score + stepv * mf[:, t]
        prev = np.where(mf[:, t] > 0, tg, prev)
    score = score + en[prev]
    alpha = st[None, :] + logits[:, 0]
    for t in range(1, T):
        nxt = alpha[:, :, None] + tr[None, :, :]
        m = nxt.max(axis=1, keepdims=True)
        nxt = np.log(np.exp(nxt - m).sum(axis=1)) + m[:, 0, :] + logits[:, t]
        alpha = np.where(mf[:, t:t + 1] > 0, nxt, alpha)
    fin = alpha + en[None, :]
    m = fin.max(axis=1, keepdims=True)
    logz = np.log(np.exp(fin - m).sum(axis=1)) + m[:, 0]
    return np.asarray((score - logz).sum(), dtype=np.float32)


def kernel(emissions, W, b, start_transitions, transitions, end_transitions,
           tags, mask):
    global LAST_RESULTS
    emissions = np.ascontiguousarray(np.asarray(emissions, dtype=np.float32))
    W = np.asarray(W, dtype=np.float32)
    b = np.asarray(b, dtype=np.float32)
    start_transitions = np.asarray(start_transitions, dtype=np.float32)
    transitions = np.asarray(transitions, dtype=np.float32)
    end_transitions = np.asarray(end_transitions, dtype=np.float32)
    tags = np.asarray(tags).astype(np.int64)
    mask = np.asarray(mask).astype(bool)

    if not mask.all():
        return _numpy_fallback(emissions, W, b, start_transitions, transitions,
                               end_transitions, tags, mask)

    from concourse.bass_utils import run_bass_kernel_spmd

    if "nc" not in _BUILT:
        _BUILT["nc"] = _build_nc()
    nc = _BUILT["nc"]

    wT_h = np.ascontiguousarray(W.reshape(2, 128, K))
    import ml_dtypes
    ehat_h = np.ascontiguousarray(
        np.exp(transitions).astype(ml_dtypes.bfloat16))
    bvec_h = np.ascontiguousarray(b.reshape(K, 1))
    estart_h = np.ascontiguousarray(np.exp(start_transitions)
                                    .astype(np.float32).reshape(K, 1))

    in_maps = []
    for c in range(NCORES):
        sh = emissions[c * BS:(c + 1) * BS]              # [BS, T, H]
        emisT_h = np.ascontiguousarray(sh.transpose(2, 1, 0)).reshape(2, 128, NT)
        tg = tags[c * BS:(c + 1) * BS]                   # [BS, T]
        oht_h = np.ascontiguousarray(
            (np.arange(K, dtype=np.int64)[:, None, None] == tg.T[None, :, :])
            .astype(np.float32).reshape(K, NT))
        in_maps.append(dict(emisT=emisT_h, oht=oht_h, wT=wT_h, ehat=ehat_h,
                            bvec=bvec_h, estart=estart_h))

    res = run_bass_kernel_spmd(nc, in_maps, list(range(NCORES)))
    LAST_RESULTS = res

    expend = np.exp(end_transitions.astype(np.float64))
    total = 0.0
    for c in range(NCORES):
        out = res.results[c]
        afinal = out["afinal"].astype(np.float64)        # [K, BS]
        shist = out["shist"].astype(np.float64).reshape(NRENORM, BS)
        gold = out["gold"].astype(np.float64)            # [K, 1]
        logz = np.log(shist).sum(axis=0) + np.log(expend @ afinal)  # [BS]
        tg = tags[c * BS:(c + 1) * BS]
        hterm = (start_transitions.astype(np.float64)[tg[:, 0]].sum()
                 + transitions.astype(np.float64)[tg[:, :-1], tg[:, 1:]].sum()
                 + end_transitions.astype(np.float64)[tg[:, -1]].sum()
                 + b.astype(np.float64)[tg].sum())
        total += gold.sum() + hterm - logz.sum()

    return np.asarray(total, dtype=np.float32)


# revision 32
# speedup vs baseline: 1.7989x; 1.3922x over previous
"""CRF log-likelihood loss kernel for Trainium2 (8 NeuronCores, Bass/Tile).

Strategy (data-parallel over batch, per sharding hint):
  - B=256 batch rows sharded 32 per core; W/b/CRF tables replicated.
  - Host pre-transposes each emissions shard to [H, T, Bs] so the device
    matmul (contract over H on partitions) needs no on-device transposes.
  - Device: logits^T[k, (t,b)] = W^T @ emisT  (PE, PSUM accumulate over 2
    h-chunks); X = exp(logits + bias) (ACT, bias fused); gold-tag logit sum
    via tensor_tensor_reduce against a host-built one-hot (DVE).
  - Forward algorithm in the linear domain: a_t[j,b] stays transposed
    [K, Bs] so each step is ONE matmul with lhsT = exp(transitions)
    augmented with a ones-column (emits column sums for free) plus ONE DVE
    multiply by X_t. Every 8 steps the state is renormalized by the sum row
    (reciprocal + ones-outer-product broadcast matmul + multiply); the norms
    are recorded and folded back in on the host.
  - Host finishes: logZ_b = sum(ln s) + ln(sum_j a_final[j,b]*exp(end_j));
    numerator = device gold-logit sum + tags-only terms (start/trans/end/bias)
    computed on host; final scalar = sum_b(score_b - logZ_b).
"""

import numpy as np

B, T, H, K = 256, 512, 256, 32
NCORES = 8
BS = B // NCORES          # 32 batch rows per core
NT = T * BS               # 16384 tokens per core
CHUNK = 2048              # tokens per DMA chunk
SUB = 512                 # tokens per matmul / X tile
NCHUNK = NT // CHUNK      # 8
NSUB = CHUNK // SUB       # 4
NXT = NT // SUB           # 32 X tiles
TS_PER_XT = SUB // BS     # 16 t-steps per X tile
RENORM = 6                # renormalize each chain's state every 6 rounds
NRENORM = 41              # per chain: measure at r=6i+2, divide at r=6i+11
NROUND = 255              # bidirectional: fwd t=1..255, bwd t=510..256

_BUILT = {}
LAST_RESULTS = None


def _build_nc(parts="all"):
    import concourse.bacc as bacc
    import concourse.tile as tile
    from concourse import mybir
    from contextlib import ExitStack

    import concourse.bass as bass
    from concourse import bass_isa

    do_bulk = parts in ("all", "bulk", "bulk_nottr", "bulk_ttr2", "bulk_mr")
    do_ttr = parts in ("all", "bulk", "bulk_ttr2", "bulk_mr")
    # TENSOR_TENSOR_REDUCE crashes this HW/FW (NRT_EXEC_UNIT_UNRECOVERABLE,
    # verified by bisection) — use mult + reduce_sum + add instead.
    ttr_mode = {"bulk_ttr2": "ttr2", "bulk": "ttr"}.get(parts, "mr")
    do_chain = parts in ("all", "chain", "chain_norenorm")
    do_renorm = parts in ("all", "chain")

    f32 = mybir.dt.float32
    bf16 = mybir.dt.bfloat16
    Exp = mybir.ActivationFunctionType.Exp
    Copy = mybir.ActivationFunctionType.Copy
    mult = mybir.AluOpType.mult
    add = mybir.AluOpType.add

    nc = bacc.Bacc("TRN2", target_bir_lowering=False, debug=False,
                   num_devices=NCORES)

    emisT = nc.declare_dram_parameter("emisT", [2, 128, NT], f32, isOutput=False)
    oht = nc.declare_dram_parameter("oht", [K, NT], f32, isOutput=False)
    wT = nc.declare_dram_parameter("wT", [2, 128, K], f32, isOutput=False)
    ehat = nc.declare_dram_parameter("ehat", [K, K], bf16, isOutput=False)
    ebwd = nc.declare_dram_parameter("ebwd", [K, K], bf16, isOutput=False)
    bvec = nc.declare_dram_parameter("bvec", [K, 1], f32, isOutput=False)
    estart = nc.declare_dram_parameter("estart", [K, 1], f32, isOutput=False)
    eend = nc.declare_dram_parameter("eend", [K, 1], f32, isOutput=False)
    amid_d = nc.declare_dram_parameter("amid", [K, BS], f32, isOutput=True)
    vmid_d = nc.declare_dram_parameter("vmid", [K, BS], f32, isOutput=True)
    shf_d = nc.declare_dram_parameter("shist", [1, NRENORM * BS], f32, isOutput=True)
    shb_d = nc.declare_dram_parameter("shistb", [1, NRENORM * BS], f32, isOutput=True)
    gold_d = nc.declare_dram_parameter("gold", [K, 1], f32, isOutput=True)

    with ExitStack() as ctx:
        tc = ctx.enter_context(tile.TileContext(nc))
        consts = ctx.enter_context(tc.tile_pool(name="consts", bufs=1))
        emis_pool = ctx.enter_context(tc.tile_pool(name="emis", bufs=3))
        oh_pool = ctx.enter_context(tc.tile_pool(name="oh", bufs=2))
        xpool = ctx.enter_context(tc.tile_pool(name="xp", bufs=NXT))
        apool = ctx.enter_context(tc.tile_pool(name="ap", bufs=16))
        tmppool = ctx.enter_context(tc.tile_pool(name="tp", bufs=2))
        rpool = ctx.enter_context(tc.tile_pool(name="rp", bufs=2))
        bcpool = ctx.enter_context(tc.tile_pool(name="bc", bufs=3))
        scrpool = ctx.enter_context(tc.tile_pool(name="scr", bufs=2))
        psum_l = ctx.enter_context(tc.tile_pool(name="pl", bufs=4, space="PSUM"))
        psum_c = ctx.enter_context(tc.tile_pool(name="pc", bufs=2, space="PSUM"))

        # constants
        w0 = consts.tile([128, K], f32)
        w1 = consts.tile([128, K], f32)
        ehat_sb = consts.tile([K, K], bf16)
        ebwd_sb = consts.tile([K, K], bf16)
        b_sb = consts.tile([K, 1], f32)
        estart_sb = consts.tile([K, 1], f32)
        eend_sb = consts.tile([K, 1], f32)
        shf_sb = consts.tile([1, NRENORM * BS], f32)
        shb_sb = consts.tile([1, NRENORM * BS], f32)
        gacc = consts.tile([K, 1], f32)
        nc.sync.dma_start(out=w0, in_=wT[0])
        nc.sync.dma_start(out=w1, in_=wT[1])
        nc.sync.dma_start(out=ehat_sb, in_=ehat[:, :])
        nc.sync.dma_start(out=ebwd_sb, in_=ebwd[:, :])
        nc.sync.dma_start(out=b_sb, in_=bvec[:, :])
        nc.sync.dma_start(out=estart_sb, in_=estart[:, :])
        nc.sync.dma_start(out=eend_sb, in_=eend[:, :])

        nc.vector.memset(gacc, 0.0)
        nc.vector.memset(shf_sb, 1.0)
        nc.vector.memset(shb_sb, 1.0)

        # ---- bulk: logits, X = exp(logits + b), gold-tag logit sum ----
        xtiles = [None] * NXT
        nttr = 0
        chunk_order = [0, 7, 1, 6, 2, 5, 3, 4]
        for c in chunk_order:
            cs, ce = c * CHUNK, (c + 1) * CHUNK
            if do_bulk:
                e0 = emis_pool.tile([128, CHUNK], f32, tag="e0")
                e1 = emis_pool.tile([128, CHUNK], f32, tag="e1")
                nc.sync.dma_start(out=e0, in_=emisT[0, :, cs:ce])
                nc.sync.dma_start(out=e1, in_=emisT[1, :, cs:ce])
                ohc = oh_pool.tile([K, CHUNK], f32, tag="ohc")
                nc.sync.dma_start(out=ohc, in_=oht[:, cs:ce])
            for s in range(NSUB):
                xt = xpool.tile([K, SUB], f32, tag="xt")
                xtiles[c * NSUB + s] = xt
                if not do_bulk:
                    nc.vector.memset(xt, 1.0)
                    continue
                pl = psum_l.tile([K, SUB], f32, tag="pl")
                nc.tensor.matmul(pl, w0, e0[:, s * SUB:(s + 1) * SUB],
                                 start=True, stop=False)
                nc.tensor.matmul(pl, w1, e1[:, s * SUB:(s + 1) * SUB],
                                 start=False, stop=True)
                nc.scalar.activation(out=xt, in_=pl, func=Exp, bias=b_sb)
                if do_ttr:
                    scr = scrpool.tile([K, SUB], f32, tag="scr")
                    ohsl = ohc[:, s * SUB:(s + 1) * SUB]
                    if ttr_mode == "ttr":
                        init = 0.0 if nttr == 0 else gacc
                        nc.vector.tensor_tensor_reduce(
                            out=scr, in0=pl, in1=ohsl,
                            scale=1.0, scalar=init, op0=mult, op1=add,
                            accum_out=gacc)
                    elif ttr_mode == "ttr2":
                        acc_c = rpool.tile([K, 1], f32, tag="acc_c")
                        nc.vector.tensor_tensor_reduce(
                            out=scr, in0=pl, in1=ohsl,
                            scale=1.0, scalar=0.0, op0=mult, op1=add,
                            accum_out=acc_c)
                        nc.vector.tensor_add(gacc, gacc, acc_c)
                    else:
                        acc_c = rpool.tile([K, 1], f32, tag="acc_c")
                        nc.vector.tensor_mul(scr, pl, ohsl)
                        nc.vector.reduce_sum(acc_c, scr,
                                             axis=mybir.AxisListType.X)
                        nc.vector.tensor_add(gacc, gacc, acc_c)
                    nttr += 1

        # ---- bidirectional chain (bf16 states, single-pass PE matmuls):
        # forward alpha from t=0 and backward beta from t=511 run as two
        # independent 255-round recurrences that interleave on PE/DVE,
        # halving the serial latency; Z = alpha_255^T E beta-part on host ----
        def xslice(t):
            return xtiles[t // TS_PER_XT][:, (t % TS_PER_XT) * BS:
                                          (t % TS_PER_XT + 1) * BS]

        a_prev = apool.tile([K, BS], bf16, tag="af")
        nc.vector.tensor_scalar(out=a_prev, in0=xslice(0),
                                scalar1=estart_sb, scalar2=None, op0=mult)
        v_prev = apool.tile([K, BS], bf16, tag="av")
        nc.vector.tensor_scalar(out=v_prev, in0=xslice(T - 1),
                                scalar1=eend_sb, scalar2=None, op0=mult)

        if do_chain:
            states = {
                "f": dict(a=a_prev, lhs=ehat_sb, slab=shf_sb, q=[], nm=0),
                "v": dict(a=v_prev, lhs=ebwd_sb, slab=shb_sb, q=[], nm=0),
            }
            for r in range(1, NROUND + 1):
                for h in ("f", "v"):
                    st = states[h]
                    t = r if h == "f" else T - 1 - r
                    xsl = xslice(t)
                    pc = psum_c.tile([K, BS], f32, tag="pc" + h)
                    nc.tensor.matmul(pc, st["lhs"], st["a"],
                                     start=True, stop=True)
                    if do_renorm and r % RENORM == 5 and st["q"]:
                        atmp = tmppool.tile([K, BS], f32, tag="tmp" + h)
                        nc.vector.tensor_mul(atmp, pc, xsl)
                        a_new = apool.tile([K, BS], bf16, tag="a" + h)
                        nc.vector.tensor_mul(a_new, atmp, st["q"].pop(0))
                    else:
                        a_new = apool.tile([K, BS], bf16, tag="a" + h)
                        nc.vector.tensor_mul(a_new, pc, xsl)
                    st["a"] = a_new
                    if (do_renorm and r % RENORM == 2
                            and st["nm"] < NRENORM):
                        slot = st["nm"]
                        st["nm"] += 1
                        bc = bcpool.tile([K, BS], f32, tag="bc" + h)
                        nc.gpsimd.partition_all_reduce(
                            bc, st["a"], channels=K,
                            reduce_op=bass_isa.ReduceOp.add)
                        nc.scalar.activation(
                            out=st["slab"][0:1, slot * BS:(slot + 1) * BS],
                            in_=bc[0:1, :], func=Copy)
                        rbc = bcpool.tile([K, BS], f32, tag="rbc" + h)
                        nc.vector.reciprocal(rbc, bc)
                        st["q"].append(rbc)
            a_prev = states["f"]["a"]
            v_prev = states["v"]["a"]

        nc.gpsimd.dma_start(out=amid_d[:, :], in_=a_prev)
        nc.gpsimd.dma_start(out=vmid_d[:, :], in_=v_prev)
        nc.sync.dma_start(out=shf_d[:, :], in_=shf_sb)
        nc.sync.dma_start(out=shb_d[:, :], in_=shb_sb)
        nc.sync.dma_start(out=gold_d[:, :], in_=gacc)

    nc.compile()
    return nc


def _numpy_fallback(emissions, W, b, start_transitions, transitions,
                    end_transitions, tags, mask):
    # Exact replication of the reference semantics (used only if mask is not
    # all-ones, which the spec's input fill guarantees never happens).
    e = emissions.astype(np.float64)
    logits = e @ W.astype(np.float64) + b.astype(np.float64)
    mf = mask.astype(np.float64)
    st = start_transitions.astype(np.float64)
    tr = transitions.astype(np.float64)
    en = end_transitions.astype(np.float64)
    Bn = logits.shape[0]
    bar = np.arange(Bn)
    first = tags[:, 0]
    score = st[first] + logits[bar, 0, first]
    prev = first.copy()
    for t in range(1, T):
        tg = tags[:, t]
        stepv = tr[prev, tg] + logits[bar, t, tg]
        score = score + stepv * mf[:, t]
        prev = np.where(mf[:, t] > 0, tg, prev)
    score = score + en[prev]
    alpha = st[None, :] + logits[:, 0]
    for t in range(1, T):
        nxt = alpha[:, :, None] + tr[None, :, :]
        m = nxt.max(axis=1, keepdims=True)
        nxt = np.log(np.exp(nxt - m).sum(axis=1)) + m[:, 0, :] + logits[:, t]
        alpha = np.where(mf[:, t:t + 1] > 0, nxt, alpha)
    fin = alpha + en[None, :]
    m = fin.max(axis=1, keepdims=True)
    logz = np.log(np.exp(fin - m).sum(axis=1)) + m[:, 0]
    return np.asarray((score - logz).sum(), dtype=np.float32)


def kernel(emissions, W, b, start_transitions, transitions, end_transitions,
           tags, mask):
    global LAST_RESULTS
    emissions = np.ascontiguousarray(np.asarray(emissions, dtype=np.float32))
    W = np.asarray(W, dtype=np.float32)
    b = np.asarray(b, dtype=np.float32)
    start_transitions = np.asarray(start_transitions, dtype=np.float32)
    transitions = np.asarray(transitions, dtype=np.float32)
    end_transitions = np.asarray(end_transitions, dtype=np.float32)
    tags = np.asarray(tags).astype(np.int64)
    mask = np.asarray(mask).astype(bool)

    if not mask.all():
        return _numpy_fallback(emissions, W, b, start_transitions, transitions,
                               end_transitions, tags, mask)

    from concourse.bass_utils import run_bass_kernel_spmd

    if "nc" not in _BUILT:
        _BUILT["nc"] = _build_nc()
    nc = _BUILT["nc"]

    wT_h = np.ascontiguousarray(W.reshape(2, 128, K))
    import ml_dtypes
    E32 = np.exp(transitions).astype(np.float32)
    ehat_h = np.ascontiguousarray(E32.astype(ml_dtypes.bfloat16))
    ebwd_h = np.ascontiguousarray(E32.T.astype(ml_dtypes.bfloat16))
    bvec_h = np.ascontiguousarray(b.reshape(K, 1))
    estart_h = np.ascontiguousarray(np.exp(start_transitions)
                                    .astype(np.float32).reshape(K, 1))
    eend_h = np.ascontiguousarray(np.exp(end_transitions)
                                  .astype(np.float32).reshape(K, 1))

    in_maps = []
    for c in range(NCORES):
        sh = emissions[c * BS:(c + 1) * BS]              # [BS, T, H]
        emisT_h = np.ascontiguousarray(sh.transpose(2, 1, 0)).reshape(2, 128, NT)
        tg = tags[c * BS:(c + 1) * BS]                   # [BS, T]
        oht_h = np.ascontiguousarray(
            (np.arange(K, dtype=np.int64)[:, None, None] == tg.T[None, :, :])
            .astype(np.float32).reshape(K, NT))
        in_maps.append(dict(emisT=emisT_h, oht=oht_h, wT=wT_h, ehat=ehat_h,
                            ebwd=ebwd_h, bvec=bvec_h, estart=estart_h,
                            eend=eend_h))

    res = run_bass_kernel_spmd(nc, in_maps, list(range(NCORES)))
    LAST_RESULTS = res

    E64 = np.exp(transitions.astype(np.float64))
    total = 0.0
    for c in range(NCORES):
        out = res.results[c]
        amid = out["amid"].astype(np.float64)            # [K, BS] alpha_255
        vmid = out["vmid"].astype(np.float64)            # [K, BS] x*beta_256
        shf = out["shist"].astype(np.float64).reshape(NRENORM, BS)
        shb = out["shistb"].astype(np.float64).reshape(NRENORM, BS)
        gold = out["gold"].astype(np.float64)            # [K, 1]
        # Z_b = alpha_255^T E (x_256*beta_256), scaled by recorded norms
        zmid = np.einsum("kb,kj,jb->b", amid, E64, vmid)
        logz = np.log(shf).sum(axis=0) + np.log(shb).sum(axis=0) + np.log(zmid)
        tg = tags[c * BS:(c + 1) * BS]
        hterm = (start_transitions.astype(np.float64)[tg[:, 0]].sum()
                 + transitions.astype(np.float64)[tg[:, :-1], tg[:, 1:]].sum()
                 + end_transitions.astype(np.float64)[tg[:, -1]].sum()
                 + b.astype(np.float64)[tg].sum())
        total += gold.sum() + hterm - logz.sum()

    return np.asarray(total, dtype=np.float32)
